# revision 16
# baseline (speedup 1.0000x reference)
"""Trainium2 Bass kernel for the CoordPooling+SFP gate module.

Computation (per batch b):
  y_pre = [sum_w x | sum_h x]                 [C, H+W]   (C=384, H=W=64)
  y  = relu((Wy/64 @ y_pre) * sy + by)        [C, 128]
  xh = relu((Wh @ y[:, :64]) * sh + bh)       [C, 64]
  xw = relu((Ww @ y[:, 64:]) * sw + bw)       [C, 64]
  z_raw[c] = sum_L y[c, :]
  z  = fc1(relu(bn1(fc0(z_raw * wscale))))    [C]
  out = x * sigmoid(xh outer xw) + x * z

Sharding: data-parallel over batch, 4 batches per core on 8 cores.

v2 design (rel-err budget 2e-2, bf16 is ~4e-3):
 - x streams in/out as bf16: halves HBM traffic (the f32 version was
   within 25% of the DMA roofline once compute was fixed).
 - reductions: TENSOR_REDUCE has no fast DVE modes (1 elem/cycle), so
   both pooling sums are binary fold trees of bf16 tensor_tensor adds,
   which hit the packed 2x_1p mode (2 elem/cycle).
 - outer-product muls go to GpSimd (on DVE the stride-0 broadcast
   operand blocks the packed mode, so DVE gains nothing from bf16
   there; GpSimd is otherwise idle).
 - combine is tensor_scalar add (4x mode) + packed TT multiply (2x)
   instead of scalar_tensor_tensor (which has no fast modes at all).
 - ScalarE keeps sigmoid + all CBR epilogues; TensorE does the (tiny)
   matmuls in bf16.
"""

import sys
import numpy as np

for _p in ("/opt/trn_rl_repo", "/root/.axon_site/_ro/trn_rl_repo"):
    if _p not in sys.path:
        sys.path.append(_p)

import ml_dtypes
import concourse.bass as bass
import concourse.tile as tile
from concourse import bacc, mybir
from concourse.bass_utils import run_bass_kernel_spmd

F32 = mybir.dt.float32
BF16 = mybir.dt.bfloat16
AF = mybir.ActivationFunctionType
OP = mybir.AluOpType

N_CORES = 8
B, C, H, W = 32, 384, 64, 64
BS = B // N_CORES          # batches per core
P = 128                    # partitions
KC = C // P                # channel chunks (3)
R = 24                     # gate bottleneck
EPS = 1e-5
HH = 32                    # h-half height for phase-2 units
NH = H // HH               # 2

# const blob layout (free-dim offsets)
_OFF_WY = 0
_OFF_WH = _OFF_WY + KC * C      # 1152
_OFF_WW = _OFF_WH + KC * C
WBLOB_F = _OFF_WW + KC * C      # 3456 (bf16 blob)
_OFF_FC0 = 0
_OFF_SY = _OFF_FC0 + KC * R     # 72
_OFF_BY = _OFF_SY + KC
_OFF_SH = _OFF_BY + KC
_OFF_BH = _OFF_SH + KC
_OFF_SW = _OFF_BH + KC
_OFF_BW = _OFF_SW + KC
_OFF_FC1B = _OFF_BW + KC
CONST_F = _OFF_FC1B + KC        # 93
_ZOFF_FC1 = 0
_ZOFF_S = KC * P                # 384
_ZOFF_B = _ZOFF_S + 1
ZCONST_F = _ZOFF_B + 1          # 386

_compiled = None


def _build():
    nc = bacc.Bacc("TRN2", target_bir_lowering=False, debug=False,
                   num_devices=N_CORES)
    x_d = nc.dram_tensor("x", [BS, C, H, W], BF16, kind="ExternalInput")
    wbl_d = nc.dram_tensor("wbl", [P, WBLOB_F], BF16, kind="ExternalInput")
    cst_d = nc.dram_tensor("cst", [P, CONST_F], F32, kind="ExternalInput")
    zcst_d = nc.dram_tensor("zcst", [R, ZCONST_F], F32, kind="ExternalInput")
    out_d = nc.dram_tensor("out", [BS, C, H, W], BF16, kind="ExternalOutput")

    with tile.TileContext(nc) as tc:
        with (
            tc.tile_pool(name="consts", bufs=1) as consts,
            tc.tile_pool(name="xp", bufs=3) as xpool,
            tc.tile_pool(name="swp", bufs=1) as swpool,
            tc.tile_pool(name="shp", bufs=1) as shpool,
            tc.tile_pool(name="ypre", bufs=2) as ypre_pool,
            tc.tile_pool(name="ysb", bufs=2) as ysb_pool,
            tc.tile_pool(name="hwp", bufs=2) as hw_pool,
            tc.tile_pool(name="zp", bufs=2) as zpool,
            tc.tile_pool(name="rpp", bufs=1) as reppool,
            tc.tile_pool(name="ltp", bufs=6) as ltpool,
            tc.tile_pool(name="psy", bufs=2, space=bass.MemorySpace.PSUM) as psy,
            tc.tile_pool(name="pshw", bufs=2, space=bass.MemorySpace.PSUM) as pshw,
            tc.tile_pool(name="psz", bufs=2, space=bass.MemorySpace.PSUM) as psz,
        ):
            wbl = consts.tile([P, WBLOB_F], BF16)
            nc.scalar.dma_start(wbl[:], wbl_d.ap())
            cst = consts.tile([P, CONST_F], F32)
            nc.scalar.dma_start(cst[:], cst_d.ap())
            zcst = consts.tile([R, ZCONST_F], F32)
            nc.scalar.dma_start(zcst[:], zcst_d.ap())

            wyT = wbl[:, _OFF_WY:_OFF_WH].rearrange("p (k o) -> p k o", k=KC)
            whT = wbl[:, _OFF_WH:_OFF_WW].rearrange("p (k o) -> p k o", k=KC)
            wwT = wbl[:, _OFF_WW:WBLOB_F].rearrange("p (k o) -> p k o", k=KC)
            fc0T = cst[:, _OFF_FC0:_OFF_SY].rearrange("p (k r) -> p k r", k=KC)
            sy_t = cst[:, _OFF_SY:_OFF_BY]
            by_t = cst[:, _OFF_BY:_OFF_SH]
            sh_t = cst[:, _OFF_SH:_OFF_BH]
            bh_t = cst[:, _OFF_BH:_OFF_SW]
            sw_t = cst[:, _OFF_SW:_OFF_BW]
            bw_t = cst[:, _OFF_BW:_OFF_FC1B]
            fc1b_t = cst[:, _OFF_FC1B:CONST_F]
            fc1T = zcst[:, _ZOFF_FC1:_ZOFF_S].rearrange("p (k o) -> p k o", k=KC)
            z2s_t = zcst[:, _ZOFF_S:_ZOFF_S + 1]
            z2b_t = zcst[:, _ZOFF_B:_ZOFF_B + 1]

            # pre-warm the sigmoid table set off the critical path
            warm = consts.tile([P, 1], F32)
            nc.scalar.activation(warm[:], cst[:, 0:1], AF.Sigmoid)

            def dma_in(b):
                x_sb = xpool.tile([P, KC, H, W], BF16, tag="x", name=f"x{b}")
                xs = x_d.ap()[b].rearrange("(k p) h w -> p k h w", p=P)
                for kc in range(KC):
                    nc.sync.dma_start(x_sb[:, kc, :, :], xs[:, kc, :, :])
                return x_sb

            def folds(x_sb, y_pre, chunked=False):
                # w-sums -> y_pre[:, :, 0:H]; all bf16 TT adds run 2x_1p
                sw = swpool.tile([P, KC, H, 32], BF16, tag="sw")
                if chunked:
                    # per-chunk L1 so the first folds start as soon as
                    # each input chunk lands (iteration-0 fill)
                    for kc in range(KC):
                        nc.vector.tensor_add(
                            sw[:, kc], x_sb[:, kc, :, 0:32],
                            x_sb[:, kc, :, 32:64])
                else:
                    nc.vector.tensor_add(sw[:], x_sb[:, :, :, 0:32],
                                         x_sb[:, :, :, 32:64])
                nc.vector.tensor_add(sw[:, :, :, 0:16], sw[:, :, :, 0:16],
                                     sw[:, :, :, 16:32])
                nc.vector.tensor_add(sw[:, :, :, 0:8], sw[:, :, :, 0:8],
                                     sw[:, :, :, 8:16])
                nc.vector.tensor_add(sw[:, :, :, 0:4], sw[:, :, :, 0:4],
                                     sw[:, :, :, 4:8])
                nc.vector.tensor_add(sw[:, :, :, 0:2], sw[:, :, :, 0:2],
                                     sw[:, :, :, 2:4])
                nc.vector.tensor_add(y_pre[:, :, 0:H], sw[:, :, :, 0],
                                     sw[:, :, :, 1])
                # h-sums -> y_pre[:, :, H:]
                sh = shpool.tile([P, KC, 32, W], BF16, tag="sh")
                if chunked:
                    for kc in range(KC):
                        nc.vector.tensor_add(
                            sh[:, kc], x_sb[:, kc, 0:32, :],
                            x_sb[:, kc, 32:64, :])
                else:
                    nc.vector.tensor_add(sh[:], x_sb[:, :, 0:32, :],
                                         x_sb[:, :, 32:64, :])
                nc.vector.tensor_add(sh[:, :, 0:16, :], sh[:, :, 0:16, :],
                                     sh[:, :, 16:32, :])
                nc.vector.tensor_add(sh[:, :, 0:8, :], sh[:, :, 0:8, :],
                                     sh[:, :, 8:16, :])
                nc.vector.tensor_add(sh[:, :, 0:4, :], sh[:, :, 0:4, :],
                                     sh[:, :, 4:8, :])
                nc.vector.tensor_add(sh[:, :, 0:2, :], sh[:, :, 0:2, :],
                                     sh[:, :, 2:4, :])
                nc.vector.tensor_add(y_pre[:, :, H:H + W], sh[:, :, 0, :],
                                     sh[:, :, 1, :])

            def chain(y_pre):
                # y = relu((Wy/64 @ y_pre)*sy + by), zraw = row-sums of y
                psum_y = psy.tile([P, KC, H + W], F32, tag="py")
                for oc in range(KC):
                    for kc in range(KC):
                        nc.tensor.matmul(
                            psum_y[:, oc, :],
                            wyT[:, kc, oc * P:(oc + 1) * P],
                            y_pre[:, kc, :],
                            start=(kc == 0), stop=(kc == KC - 1))
                y_sb = ysb_pool.tile([P, KC, H + W], BF16, tag="y")
                zraw = zpool.tile([P, KC, 1], F32, tag="zraw")
                for oc in range(KC):
                    nc.scalar.activation(
                        y_sb[:, oc, :], psum_y[:, oc, :], AF.Relu,
                        bias=by_t[:, oc:oc + 1], scale=sy_t[:, oc:oc + 1],
                        accum_out=zraw[:, oc, :])

                # z chain (tiny, fp32)
                psum_z = psz.tile([R, 1], F32, tag="pz")
                for kc in range(KC):
                    nc.tensor.matmul(
                        psum_z[:], fc0T[:, kc, :], zraw[:, kc, :],
                        start=(kc == 0), stop=(kc == KC - 1))
                z2 = zpool.tile([R, 1], F32, tag="z2")
                nc.scalar.activation(z2[:], psum_z[:], AF.Relu,
                                     bias=z2b_t[:], scale=z2s_t[:])
                # z3 stays in PSUM; the combine's tensor_scalar adds
                # psum_z3 and fc1b as its two scalar operands, so no
                # separate epilogue pass is needed
                psum_z3 = psz.tile([P, KC], F32, tag="pz3")
                for oc in range(KC):
                    nc.tensor.matmul(
                        psum_z3[:, oc:oc + 1], fc1T[:, oc, :], z2[:],
                        start=True, stop=True)
                z3 = psum_z3

                # xh / xw
                psum_hw = pshw.tile([P, KC, H + W], F32, tag="phw")
                for oc in range(KC):
                    for kc in range(KC):
                        nc.tensor.matmul(
                            psum_hw[:, oc, 0:H],
                            whT[:, kc, oc * P:(oc + 1) * P],
                            y_sb[:, kc, 0:H],
                            start=(kc == 0), stop=(kc == KC - 1))
                    for kc in range(KC):
                        nc.tensor.matmul(
                            psum_hw[:, oc, H:H + W],
                            wwT[:, kc, oc * P:(oc + 1) * P],
                            y_sb[:, kc, H:H + W],
                            start=(kc == 0), stop=(kc == KC - 1))
                xh = hw_pool.tile([P, KC, H], BF16, tag="xh")
                xw = hw_pool.tile([P, KC, W], BF16, tag="xw")
                for oc in range(KC):
                    nc.scalar.activation(
                        xh[:, oc, :], psum_hw[:, oc, 0:H], AF.Relu,
                        bias=bh_t[:, oc:oc + 1], scale=sh_t[:, oc:oc + 1])
                    nc.scalar.activation(
                        xw[:, oc, :], psum_hw[:, oc, H:H + W], AF.Relu,
                        bias=bw_t[:, oc:oc + 1], scale=sw_t[:, oc:oc + 1])
                return y_sb, zraw, z2, z3, xh, xw

            def emit_gate(st):
                # ScalarE materializes the xh replica (ACT reads the
                # stride-0 broadcast AP at its usual 1 elem/cycle), so
                # the outer-product multiply becomes a fully packed
                # bf16 TT that runs in 2x mode on DVE. GpSimd stays
                # idle on purpose: any GpSimd activity degrades DVE
                # 2-port instructions via the shared SBUF port.
                x_sb, xh, xw, z3, b = st
                reps = {}
                for oc in range(KC):
                    rep = reppool.tile([P, H, W], BF16, tag=f"rep{oc}",
                                       name=f"rep{b}_{oc}")
                    nc.scalar.copy(
                        rep[:],
                        xh[:, oc, :].unsqueeze(2).broadcast_to([P, H, W]))
                    reps[oc] = rep
                ls = {}
                for oc in range(KC):
                    l_t = ltpool.tile([P, H, W], BF16, tag="l",
                                      name=f"l{b}_{oc}")
                    nc.vector.tensor_mul(
                        l_t[:], reps[oc][:],
                        xw[:, oc, :].unsqueeze(1)
                          .broadcast_to([P, H, W]))
                    ls[oc] = l_t
                for oc in range(KC):
                    nc.scalar.activation(ls[oc][:], ls[oc][:], AF.Sigmoid)
                return ls

            def emit_combine(st, ls, split_out=False):
                # x = (sigmoid + z) * x in place, then store
                x_sb, xh, xw, z3, b = st
                for oc in range(KC):
                    l_t = ls[oc]
                    nc.vector.tensor_scalar(
                        l_t[:], l_t[:], z3[:, oc:oc + 1],
                        fc1b_t[:, oc:oc + 1], op0=OP.add, op1=OP.add)
                    nc.vector.tensor_mul(
                        x_sb[:, oc, :, :], l_t[:], x_sb[:, oc, :, :])
                    eng = nc.scalar if (split_out and oc == 1) else nc.sync
                    eng.dma_start(
                        out_d.ap()[b, oc * P:(oc + 1) * P],
                        x_sb[:, oc, :, :])

            prev = None
            prev_ls = None
            x_cur = dma_in(0)
            for b in range(BS):
                x_nxt = dma_in(b + 1) if b + 1 < BS else None
                y_pre = ypre_pool.tile([P, KC, H + W], BF16, tag="ypre")
                folds(x_cur, y_pre, chunked=(b == 0))
                if prev is not None:
                    emit_combine(prev, prev_ls)
                _, _, _, z3, xh, xw = chain(y_pre)
                prev = (x_cur, xh, xw, z3, b)
                prev_ls = emit_gate(prev)
                x_cur = x_nxt
            emit_combine(prev, prev_ls, split_out=True)

    nc.compile()
    return nc


def _pack_consts(Wy, gy, by, Wh, gh, bh, Ww, gw, bw,
                 fc0_w, fc0_b, bn1_g, bn1_b, fc1_w, fc1_b):
    inv = 1.0 / np.sqrt(1.0 + EPS)

    def chunked_T(w):
        # [out, in] -> lhsT tile [p, kc, out]
        return np.ascontiguousarray(
            w.T.reshape(KC, P, C).transpose(1, 0, 2))

    def lanes(v):
        # [C] -> [p, kc]
        return np.ascontiguousarray(v.reshape(KC, P).T)

    wbl = np.empty((P, WBLOB_F), np.float32)
    wbl[:, _OFF_WY:_OFF_WH] = chunked_T(Wy / 64.0).reshape(P, KC * C)
    wbl[:, _OFF_WH:_OFF_WW] = chunked_T(Wh).reshape(P, KC * C)
    wbl[:, _OFF_WW:WBLOB_F] = chunked_T(Ww).reshape(P, KC * C)
    cst = np.empty((P, CONST_F), np.float32)
    # wavelet level-i scale per channel chunk, folded into fc0
    wscale = np.repeat(2.0 ** (np.arange(1, KC + 1) / 2.0) / (H + W), P)
    fc0T_s = (fc0_w * wscale[None, :]).T        # [C, R]
    cst[:, _OFF_FC0:_OFF_SY] = fc0T_s.reshape(KC, P, R).transpose(1, 0, 2) \
                                     .reshape(P, KC * R)
    cst[:, _OFF_SY:_OFF_BY] = lanes(gy * inv)
    cst[:, _OFF_BY:_OFF_SH] = lanes(by)
    cst[:, _OFF_SH:_OFF_BH] = lanes(gh * inv)
    cst[:, _OFF_BH:_OFF_SW] = lanes(bh)
    cst[:, _OFF_SW:_OFF_BW] = lanes(gw * inv)
    cst[:, _OFF_BW:_OFF_FC1B] = lanes(bw)
    cst[:, _OFF_FC1B:CONST_F] = lanes(fc1_b)

    zcst = np.empty((R, ZCONST_F), np.float32)
    zcst[:, _ZOFF_FC1:_ZOFF_S] = fc1_w.T.reshape(R, KC * P)
    z2s = bn1_g * inv
    zcst[:, _ZOFF_S] = z2s
    zcst[:, _ZOFF_B] = fc0_b * z2s + bn1_b
    return wbl.astype(ml_dtypes.bfloat16), cst, zcst


def _get_compiled():
    global _compiled
    if _compiled is None:
        _compiled = _build()
    return _compiled


def kernel(x, Wy, gy, by, Wh, gh, bh, Ww, gw, bw,
           fc0_w, fc0_b, bn1_g, bn1_b, fc1_w, fc1_b,
           _trace=False, _trace_kwargs=None):
    nc = _get_compiled()
    wbl, cst, zcst = _pack_consts(
        np.asarray(Wy, np.float32), np.asarray(gy, np.float32),
        np.asarray(by, np.float32), np.asarray(Wh, np.float32),
        np.asarray(gh, np.float32), np.asarray(bh, np.float32),
        np.asarray(Ww, np.float32), np.asarray(gw, np.float32),
        np.asarray(bw, np.float32), np.asarray(fc0_w, np.float32),
        np.asarray(fc0_b, np.float32), np.asarray(bn1_g, np.float32),
        np.asarray(bn1_b, np.float32), np.asarray(fc1_w, np.float32),
        np.asarray(fc1_b, np.float32))
    x = np.asarray(x, np.float32).astype(ml_dtypes.bfloat16)
    x = np.ascontiguousarray(x)
    in_maps = [
        {"x": x[i * BS:(i + 1) * BS], "wbl": wbl, "cst": cst, "zcst": zcst}
        for i in range(N_CORES)
    ]
    res = run_bass_kernel_spmd(
        nc, in_maps, list(range(N_CORES)),
        trace=_trace, **(_trace_kwargs or {}))
    out = np.concatenate(
        [res.results[i]["out"] for i in range(N_CORES)],
        axis=0).astype(np.float32)
    if _trace:
        return out, res
    return out


# revision 19
# speedup vs baseline: 1.0128x; 1.0128x over previous
"""Trainium2 Bass kernel for the CoordPooling+SFP gate module.

Computation (per batch b):
  y_pre = [sum_w x | sum_h x]                 [C, H+W]   (C=384, H=W=64)
  y  = relu((Wy/64 @ y_pre) * sy + by)        [C, 128]
  xh = relu((Wh @ y[:, :64]) * sh + bh)       [C, 64]
  xw = relu((Ww @ y[:, 64:]) * sw + bw)       [C, 64]
  z_raw[c] = sum_L y[c, :]
  z  = fc1(relu(bn1(fc0(z_raw * wscale))))    [C]
  out = x * sigmoid(xh outer xw) + x * z

Sharding: data-parallel over batch, 4 batches per core on 8 cores.

v2 design (rel-err budget 2e-2, bf16 is ~4e-3):
 - x streams in/out as bf16: halves HBM traffic (the f32 version was
   within 25% of the DMA roofline once compute was fixed).
 - reductions: TENSOR_REDUCE has no fast DVE modes (1 elem/cycle), so
   both pooling sums are binary fold trees of bf16 tensor_tensor adds,
   which hit the packed 2x_1p mode (2 elem/cycle).
 - outer-product muls go to GpSimd (on DVE the stride-0 broadcast
   operand blocks the packed mode, so DVE gains nothing from bf16
   there; GpSimd is otherwise idle).
 - combine is tensor_scalar add (4x mode) + packed TT multiply (2x)
   instead of scalar_tensor_tensor (which has no fast modes at all).
 - ScalarE keeps sigmoid + all CBR epilogues; TensorE does the (tiny)
   matmuls in bf16.
"""

import sys
import numpy as np

for _p in ("/opt/trn_rl_repo", "/root/.axon_site/_ro/trn_rl_repo"):
    if _p not in sys.path:
        sys.path.append(_p)

import ml_dtypes
import concourse.bass as bass
import concourse.tile as tile
from concourse import bacc, mybir
from concourse.bass_utils import run_bass_kernel_spmd

F32 = mybir.dt.float32
BF16 = mybir.dt.bfloat16
AF = mybir.ActivationFunctionType
OP = mybir.AluOpType

N_CORES = 8
B, C, H, W = 32, 384, 64, 64
BS = B // N_CORES          # batches per core
P = 128                    # partitions
KC = C // P                # channel chunks (3)
R = 24                     # gate bottleneck
EPS = 1e-5
HH = 32                    # h-half height for phase-2 units
NH = H // HH               # 2

# const blob layout (free-dim offsets)
_OFF_WY = 0
_OFF_WH = _OFF_WY + KC * C      # 1152
_OFF_WW = _OFF_WH + KC * C
WBLOB_F = _OFF_WW + KC * C      # 3456 (bf16 blob)
_OFF_FC0 = 0
_OFF_SY = _OFF_FC0 + KC * R     # 72
_OFF_BY = _OFF_SY + KC
_OFF_SH = _OFF_BY + KC
_OFF_BH = _OFF_SH + KC
_OFF_SW = _OFF_BH + KC
_OFF_BW = _OFF_SW + KC
_OFF_FC1B = _OFF_BW + KC
CONST_F = _OFF_FC1B + KC        # 93
_ZOFF_FC1 = 0
_ZOFF_S = KC * P                # 384
_ZOFF_B = _ZOFF_S + 1
ZCONST_F = _ZOFF_B + 1          # 386

_compiled = None


def _build():
    nc = bacc.Bacc("TRN2", target_bir_lowering=False, debug=False,
                   num_devices=N_CORES)
    x_d = nc.dram_tensor("x", [BS, C, H, W], BF16, kind="ExternalInput")
    wbl_d = nc.dram_tensor("wbl", [P, WBLOB_F], BF16, kind="ExternalInput")
    cst_d = nc.dram_tensor("cst", [P, CONST_F], F32, kind="ExternalInput")
    zcst_d = nc.dram_tensor("zcst", [R, ZCONST_F], F32, kind="ExternalInput")
    out_d = nc.dram_tensor("out", [BS, C, H, W], BF16, kind="ExternalOutput")

    with tile.TileContext(nc) as tc:
        with (
            tc.tile_pool(name="consts", bufs=1) as consts,
            tc.tile_pool(name="xp", bufs=3) as xpool,
            tc.tile_pool(name="swp", bufs=1) as swpool,
            tc.tile_pool(name="shp", bufs=1) as shpool,
            tc.tile_pool(name="ypre", bufs=2) as ypre_pool,
            tc.tile_pool(name="ysb", bufs=2) as ysb_pool,
            tc.tile_pool(name="hwp", bufs=2) as hw_pool,
            tc.tile_pool(name="zp", bufs=2) as zpool,
            tc.tile_pool(name="rpp", bufs=1) as reppool,
            tc.tile_pool(name="ltp", bufs=6) as ltpool,
            tc.tile_pool(name="psy", bufs=2, space=bass.MemorySpace.PSUM) as psy,
            tc.tile_pool(name="pshw", bufs=2, space=bass.MemorySpace.PSUM) as pshw,
            tc.tile_pool(name="psz", bufs=2, space=bass.MemorySpace.PSUM) as psz,
        ):
            wbl = consts.tile([P, WBLOB_F], BF16)
            nc.scalar.dma_start(wbl[:], wbl_d.ap())
            cst = consts.tile([P, CONST_F], F32)
            nc.scalar.dma_start(cst[:], cst_d.ap())
            zcst = consts.tile([R, ZCONST_F], F32)
            nc.scalar.dma_start(zcst[:], zcst_d.ap())

            wyT = wbl[:, _OFF_WY:_OFF_WH].rearrange("p (k o) -> p k o", k=KC)
            whT = wbl[:, _OFF_WH:_OFF_WW].rearrange("p (k o) -> p k o", k=KC)
            wwT = wbl[:, _OFF_WW:WBLOB_F].rearrange("p (k o) -> p k o", k=KC)
            fc0T = cst[:, _OFF_FC0:_OFF_SY].rearrange("p (k r) -> p k r", k=KC)
            sy_t = cst[:, _OFF_SY:_OFF_BY]
            by_t = cst[:, _OFF_BY:_OFF_SH]
            sh_t = cst[:, _OFF_SH:_OFF_BH]
            bh_t = cst[:, _OFF_BH:_OFF_SW]
            sw_t = cst[:, _OFF_SW:_OFF_BW]
            bw_t = cst[:, _OFF_BW:_OFF_FC1B]
            fc1b_t = cst[:, _OFF_FC1B:CONST_F]
            fc1T = zcst[:, _ZOFF_FC1:_ZOFF_S].rearrange("p (k o) -> p k o", k=KC)
            z2s_t = zcst[:, _ZOFF_S:_ZOFF_S + 1]
            z2b_t = zcst[:, _ZOFF_B:_ZOFF_B + 1]

            # pre-warm the sigmoid table set off the critical path
            warm = consts.tile([P, 1], F32)
            nc.scalar.activation(warm[:], cst[:, 0:1], AF.Sigmoid)

            def dma_in(b):
                x_sb = xpool.tile([P, KC, H, W], BF16, tag="x", name=f"x{b}")
                xs = x_d.ap()[b].rearrange("(k p) h w -> p k h w", p=P)
                for kc in range(KC):
                    nc.sync.dma_start(x_sb[:, kc, :, :], xs[:, kc, :, :])
                return x_sb

            def folds(x_sb, y_pre, chunked=False):
                # w-sums -> y_pre[:, :, 0:H]; all bf16 TT adds run 2x_1p
                sw = swpool.tile([P, KC, H, 32], BF16, tag="sw")
                if chunked:
                    # per-chunk L1 so the first folds start as soon as
                    # each input chunk lands (iteration-0 fill)
                    for kc in range(KC):
                        nc.vector.tensor_add(
                            sw[:, kc], x_sb[:, kc, :, 0:32],
                            x_sb[:, kc, :, 32:64])
                else:
                    nc.vector.tensor_add(sw[:], x_sb[:, :, :, 0:32],
                                         x_sb[:, :, :, 32:64])
                nc.vector.tensor_add(sw[:, :, :, 0:16], sw[:, :, :, 0:16],
                                     sw[:, :, :, 16:32])
                nc.vector.tensor_add(sw[:, :, :, 0:8], sw[:, :, :, 0:8],
                                     sw[:, :, :, 8:16])
                nc.vector.tensor_add(sw[:, :, :, 0:4], sw[:, :, :, 0:4],
                                     sw[:, :, :, 4:8])
                nc.vector.tensor_add(sw[:, :, :, 0:2], sw[:, :, :, 0:2],
                                     sw[:, :, :, 2:4])
                nc.vector.tensor_add(y_pre[:, :, 0:H], sw[:, :, :, 0],
                                     sw[:, :, :, 1])
                # h-sums -> y_pre[:, :, H:]
                sh = shpool.tile([P, KC, 32, W], BF16, tag="sh")
                if chunked:
                    for kc in range(KC):
                        nc.vector.tensor_add(
                            sh[:, kc], x_sb[:, kc, 0:32, :],
                            x_sb[:, kc, 32:64, :])
                else:
                    nc.vector.tensor_add(sh[:], x_sb[:, :, 0:32, :],
                                         x_sb[:, :, 32:64, :])
                nc.vector.tensor_add(sh[:, :, 0:16, :], sh[:, :, 0:16, :],
                                     sh[:, :, 16:32, :])
                nc.vector.tensor_add(sh[:, :, 0:8, :], sh[:, :, 0:8, :],
                                     sh[:, :, 8:16, :])
                nc.vector.tensor_add(sh[:, :, 0:4, :], sh[:, :, 0:4, :],
                                     sh[:, :, 4:8, :])
                nc.vector.tensor_add(sh[:, :, 0:2, :], sh[:, :, 0:2, :],
                                     sh[:, :, 2:4, :])
                nc.vector.tensor_add(y_pre[:, :, H:H + W], sh[:, :, 0, :],
                                     sh[:, :, 1, :])

            def chain(y_pre):
                # y = relu((Wy/64 @ y_pre)*sy + by), zraw = row-sums of y
                psum_y = psy.tile([P, KC, H + W], F32, tag="py")
                for oc in range(KC):
                    for kc in range(KC):
                        nc.tensor.matmul(
                            psum_y[:, oc, :],
                            wyT[:, kc, oc * P:(oc + 1) * P],
                            y_pre[:, kc, :],
                            start=(kc == 0), stop=(kc == KC - 1))
                y_sb = ysb_pool.tile([P, KC, H + W], BF16, tag="y")
                zraw = zpool.tile([P, KC, 1], F32, tag="zraw")
                for oc in range(KC):
                    nc.scalar.activation(
                        y_sb[:, oc, :], psum_y[:, oc, :], AF.Relu,
                        bias=by_t[:, oc:oc + 1], scale=sy_t[:, oc:oc + 1],
                        accum_out=zraw[:, oc, :])

                # z chain (tiny, fp32)
                psum_z = psz.tile([R, 1], F32, tag="pz")
                for kc in range(KC):
                    nc.tensor.matmul(
                        psum_z[:], fc0T[:, kc, :], zraw[:, kc, :],
                        start=(kc == 0), stop=(kc == KC - 1))
                z2 = zpool.tile([R, 1], F32, tag="z2")
                nc.scalar.activation(z2[:], psum_z[:], AF.Relu,
                                     bias=z2b_t[:], scale=z2s_t[:])
                psum_z3 = psz.tile([P, KC], F32, tag="pz3")
                for oc in range(KC):
                    nc.tensor.matmul(
                        psum_z3[:, oc:oc + 1], fc1T[:, oc, :], z2[:],
                        start=True, stop=True)
                z3 = zpool.tile([P, KC, 1], F32, tag="z3")
                for oc in range(KC):
                    nc.scalar.activation(
                        z3[:, oc, :], psum_z3[:, oc:oc + 1], AF.Identity,
                        bias=fc1b_t[:, oc:oc + 1])

                # xh / xw
                psum_hw = pshw.tile([P, KC, H + W], F32, tag="phw")
                for oc in range(KC):
                    for kc in range(KC):
                        nc.tensor.matmul(
                            psum_hw[:, oc, 0:H],
                            whT[:, kc, oc * P:(oc + 1) * P],
                            y_sb[:, kc, 0:H],
                            start=(kc == 0), stop=(kc == KC - 1))
                    for kc in range(KC):
                        nc.tensor.matmul(
                            psum_hw[:, oc, H:H + W],
                            wwT[:, kc, oc * P:(oc + 1) * P],
                            y_sb[:, kc, H:H + W],
                            start=(kc == 0), stop=(kc == KC - 1))
                xh = hw_pool.tile([P, KC, H], BF16, tag="xh")
                xw = hw_pool.tile([P, KC, W], BF16, tag="xw")
                for oc in range(KC):
                    nc.scalar.activation(
                        xh[:, oc, :], psum_hw[:, oc, 0:H], AF.Relu,
                        bias=bh_t[:, oc:oc + 1], scale=sh_t[:, oc:oc + 1])
                    nc.scalar.activation(
                        xw[:, oc, :], psum_hw[:, oc, H:H + W], AF.Relu,
                        bias=bw_t[:, oc:oc + 1], scale=sw_t[:, oc:oc + 1])
                return y_sb, zraw, z2, z3, xh, xw

            def emit_gate(st):
                # ScalarE materializes the xh replica (ACT reads the
                # stride-0 broadcast AP at its usual 1 elem/cycle), so
                # the outer-product multiply becomes a fully packed
                # bf16 TT that runs in 2x mode on DVE. GpSimd stays
                # idle on purpose: any GpSimd activity degrades DVE
                # 2-port instructions via the shared SBUF port.
                x_sb, xh, xw, z3, b = st
                reps = {}
                for oc in range(KC):
                    rep = reppool.tile([P, H, W], BF16, tag=f"rep{oc}",
                                       name=f"rep{b}_{oc}")
                    nc.scalar.copy(
                        rep[:],
                        xh[:, oc, :].unsqueeze(2).broadcast_to([P, H, W]))
                    reps[oc] = rep
                ls = {}
                for oc in range(KC):
                    l_t = ltpool.tile([P, H, W], BF16, tag="l",
                                      name=f"l{b}_{oc}")
                    for hh in range(NH):
                        h0 = hh * HH
                        nc.vector.tensor_mul(
                            l_t[:, h0:h0 + HH, :],
                            reps[oc][:, h0:h0 + HH, :],
                            xw[:, oc, :].unsqueeze(1)
                              .broadcast_to([P, HH, W]))
                    ls[oc] = l_t
                for oc in range(KC):
                    nc.scalar.activation(ls[oc][:], ls[oc][:], AF.Sigmoid)
                return ls

            def emit_combine(st, ls, split_out=False):
                # x = (sigmoid + z) * x in place, then store
                x_sb, xh, xw, z3, b = st
                for oc in range(KC):
                    l_t = ls[oc]
                    nc.vector.tensor_scalar_add(l_t[:], l_t[:],
                                                z3[:, oc, :])
                    nc.vector.tensor_mul(
                        x_sb[:, oc, :, :], l_t[:], x_sb[:, oc, :, :])
                    eng = nc.scalar if (split_out and oc == 1) else nc.sync
                    eng.dma_start(
                        out_d.ap()[b, oc * P:(oc + 1) * P],
                        x_sb[:, oc, :, :])

            prev = None
            prev_ls = None
            x_cur = dma_in(0)
            for b in range(BS):
                x_nxt = dma_in(b + 1) if b + 1 < BS else None
                y_pre = ypre_pool.tile([P, KC, H + W], BF16, tag="ypre")
                folds(x_cur, y_pre, chunked=(b == 0))
                if prev is not None:
                    emit_combine(prev, prev_ls)
                _, _, _, z3, xh, xw = chain(y_pre)
                prev = (x_cur, xh, xw, z3, b)
                prev_ls = emit_gate(prev)
                x_cur = x_nxt
            emit_combine(prev, prev_ls, split_out=True)

    nc.compile()
    return nc


def _pack_consts(Wy, gy, by, Wh, gh, bh, Ww, gw, bw,
                 fc0_w, fc0_b, bn1_g, bn1_b, fc1_w, fc1_b):
    inv = 1.0 / np.sqrt(1.0 + EPS)

    def chunked_T(w):
        # [out, in] -> lhsT tile [p, kc, out]
        return np.ascontiguousarray(
            w.T.reshape(KC, P, C).transpose(1, 0, 2))

    def lanes(v):
        # [C] -> [p, kc]
        return np.ascontiguousarray(v.reshape(KC, P).T)

    wbl = np.empty((P, WBLOB_F), np.float32)
    wbl[:, _OFF_WY:_OFF_WH] = chunked_T(Wy / 64.0).reshape(P, KC * C)
    wbl[:, _OFF_WH:_OFF_WW] = chunked_T(Wh).reshape(P, KC * C)
    wbl[:, _OFF_WW:WBLOB_F] = chunked_T(Ww).reshape(P, KC * C)
    cst = np.empty((P, CONST_F), np.float32)
    # wavelet level-i scale per channel chunk, folded into fc0
    wscale = np.repeat(2.0 ** (np.arange(1, KC + 1) / 2.0) / (H + W), P)
    fc0T_s = (fc0_w * wscale[None, :]).T        # [C, R]
    cst[:, _OFF_FC0:_OFF_SY] = fc0T_s.reshape(KC, P, R).transpose(1, 0, 2) \
                                     .reshape(P, KC * R)
    cst[:, _OFF_SY:_OFF_BY] = lanes(gy * inv)
    cst[:, _OFF_BY:_OFF_SH] = lanes(by)
    cst[:, _OFF_SH:_OFF_BH] = lanes(gh * inv)
    cst[:, _OFF_BH:_OFF_SW] = lanes(bh)
    cst[:, _OFF_SW:_OFF_BW] = lanes(gw * inv)
    cst[:, _OFF_BW:_OFF_FC1B] = lanes(bw)
    cst[:, _OFF_FC1B:CONST_F] = lanes(fc1_b)

    zcst = np.empty((R, ZCONST_F), np.float32)
    zcst[:, _ZOFF_FC1:_ZOFF_S] = fc1_w.T.reshape(R, KC * P)
    z2s = bn1_g * inv
    zcst[:, _ZOFF_S] = z2s
    zcst[:, _ZOFF_B] = fc0_b * z2s + bn1_b
    return wbl.astype(ml_dtypes.bfloat16), cst, zcst


def _get_compiled():
    global _compiled
    if _compiled is None:
        _compiled = _build()
    return _compiled


def kernel(x, Wy, gy, by, Wh, gh, bh, Ww, gw, bw,
           fc0_w, fc0_b, bn1_g, bn1_b, fc1_w, fc1_b,
           _trace=False, _trace_kwargs=None):
    nc = _get_compiled()
    wbl, cst, zcst = _pack_consts(
        np.asarray(Wy, np.float32), np.asarray(gy, np.float32),
        np.asarray(by, np.float32), np.asarray(Wh, np.float32),
        np.asarray(gh, np.float32), np.asarray(bh, np.float32),
        np.asarray(Ww, np.float32), np.asarray(gw, np.float32),
        np.asarray(bw, np.float32), np.asarray(fc0_w, np.float32),
        np.asarray(fc0_b, np.float32), np.asarray(bn1_g, np.float32),
        np.asarray(bn1_b, np.float32), np.asarray(fc1_w, np.float32),
        np.asarray(fc1_b, np.float32))
    x = np.asarray(x, np.float32).astype(ml_dtypes.bfloat16)
    x = np.ascontiguousarray(x)
    in_maps = [
        {"x": x[i * BS:(i + 1) * BS], "wbl": wbl, "cst": cst, "zcst": zcst}
        for i in range(N_CORES)
    ]
    res = run_bass_kernel_spmd(
        nc, in_maps, list(range(N_CORES)),
        trace=_trace, **(_trace_kwargs or {}))
    out = np.concatenate(
        [res.results[i]["out"] for i in range(N_CORES)],
        axis=0).astype(np.float32)
    if _trace:
        return out, res
    return out


# revision 24
# speedup vs baseline: 1.0135x; 1.0007x over previous
"""Trainium2 Bass kernel for the CoordPooling+SFP gate module.

Computation (per batch b):
  y_pre = [sum_w x | sum_h x]                 [C, H+W]   (C=384, H=W=64)
  y  = relu((Wy/64 @ y_pre) * sy + by)        [C, 128]
  xh = relu((Wh @ y[:, :64]) * sh + bh)       [C, 64]
  xw = relu((Ww @ y[:, 64:]) * sw + bw)       [C, 64]
  z_raw[c] = sum_L y[c, :]
  z  = fc1(relu(bn1(fc0(z_raw * wscale))))    [C]
  out = x * sigmoid(xh outer xw) + x * z

Sharding: data-parallel over batch, 4 batches per core on 8 cores.

v2 design (rel-err budget 2e-2, bf16 is ~4e-3):
 - x streams in/out as bf16: halves HBM traffic (the f32 version was
   within 25% of the DMA roofline once compute was fixed).
 - reductions: TENSOR_REDUCE has no fast DVE modes (1 elem/cycle), so
   both pooling sums are binary fold trees of bf16 tensor_tensor adds,
   which hit the packed 2x_1p mode (2 elem/cycle).
 - outer-product muls go to GpSimd (on DVE the stride-0 broadcast
   operand blocks the packed mode, so DVE gains nothing from bf16
   there; GpSimd is otherwise idle).
 - combine is tensor_scalar add (4x mode) + packed TT multiply (2x)
   instead of scalar_tensor_tensor (which has no fast modes at all).
 - ScalarE keeps sigmoid + all CBR epilogues; TensorE does the (tiny)
   matmuls in bf16.
"""

import sys
import numpy as np

for _p in ("/opt/trn_rl_repo", "/root/.axon_site/_ro/trn_rl_repo"):
    if _p not in sys.path:
        sys.path.append(_p)

import ml_dtypes
import concourse.bass as bass
import concourse.tile as tile
from concourse import bacc, mybir
from concourse.bass_utils import run_bass_kernel_spmd

F32 = mybir.dt.float32
BF16 = mybir.dt.bfloat16
AF = mybir.ActivationFunctionType
OP = mybir.AluOpType

N_CORES = 8
B, C, H, W = 32, 384, 64, 64
BS = B // N_CORES          # batches per core
P = 128                    # partitions
KC = C // P                # channel chunks (3)
R = 24                     # gate bottleneck
EPS = 1e-5
HH = 32                    # h-half height for phase-2 units
NH = H // HH               # 2

# const blob layout (free-dim offsets)
_OFF_WY = 0
_OFF_WH = _OFF_WY + KC * C      # 1152
_OFF_WW = _OFF_WH + KC * C
WBLOB_F = _OFF_WW + KC * C      # 3456 (bf16 blob)
_OFF_FC0 = 0
_OFF_SY = _OFF_FC0 + KC * R     # 72
_OFF_BY = _OFF_SY + KC
_OFF_SH = _OFF_BY + KC
_OFF_BH = _OFF_SH + KC
_OFF_SW = _OFF_BH + KC
_OFF_BW = _OFF_SW + KC
_OFF_FC1B = _OFF_BW + KC
CONST_F = _OFF_FC1B + KC        # 93
_ZOFF_FC1 = 0
_ZOFF_S = KC * P                # 384
_ZOFF_B = _ZOFF_S + 1
ZCONST_F = _ZOFF_B + 1          # 386

_compiled = None


def _build():
    nc = bacc.Bacc("TRN2", target_bir_lowering=False, debug=False,
                   num_devices=N_CORES)
    x_d = nc.dram_tensor("x", [BS, C, H, W], BF16, kind="ExternalInput")
    wbl_d = nc.dram_tensor("wbl", [P, WBLOB_F], BF16, kind="ExternalInput")
    cst_d = nc.dram_tensor("cst", [P, CONST_F], F32, kind="ExternalInput")
    zcst_d = nc.dram_tensor("zcst", [R, ZCONST_F], F32, kind="ExternalInput")
    out_d = nc.dram_tensor("out", [BS, C, H, W], BF16, kind="ExternalOutput")

    with tile.TileContext(nc) as tc:
        with (
            tc.tile_pool(name="consts", bufs=1) as consts,
            tc.tile_pool(name="xp", bufs=3) as xpool,
            tc.tile_pool(name="swp", bufs=1) as swpool,
            tc.tile_pool(name="shp", bufs=1) as shpool,
            tc.tile_pool(name="ypre", bufs=2) as ypre_pool,
            tc.tile_pool(name="ysb", bufs=2) as ysb_pool,
            tc.tile_pool(name="hwp", bufs=2) as hw_pool,
            tc.tile_pool(name="zp", bufs=2) as zpool,
            tc.tile_pool(name="rpp", bufs=1) as reppool,
            tc.tile_pool(name="ltp", bufs=6) as ltpool,
            tc.tile_pool(name="psy", bufs=2, space=bass.MemorySpace.PSUM) as psy,
            tc.tile_pool(name="pshw", bufs=2, space=bass.MemorySpace.PSUM) as pshw,
            tc.tile_pool(name="psz", bufs=2, space=bass.MemorySpace.PSUM) as psz,
        ):
            wbl = consts.tile([P, WBLOB_F], BF16)
            nc.scalar.dma_start(wbl[:], wbl_d.ap())
            cst = consts.tile([P, CONST_F], F32)
            nc.scalar.dma_start(cst[:], cst_d.ap())
            zcst = consts.tile([R, ZCONST_F], F32)
            nc.scalar.dma_start(zcst[:], zcst_d.ap())

            wyT = wbl[:, _OFF_WY:_OFF_WH].rearrange("p (k o) -> p k o", k=KC)
            whT = wbl[:, _OFF_WH:_OFF_WW].rearrange("p (k o) -> p k o", k=KC)
            wwT = wbl[:, _OFF_WW:WBLOB_F].rearrange("p (k o) -> p k o", k=KC)
            fc0T = cst[:, _OFF_FC0:_OFF_SY].rearrange("p (k r) -> p k r", k=KC)
            sy_t = cst[:, _OFF_SY:_OFF_BY]
            by_t = cst[:, _OFF_BY:_OFF_SH]
            sh_t = cst[:, _OFF_SH:_OFF_BH]
            bh_t = cst[:, _OFF_BH:_OFF_SW]
            sw_t = cst[:, _OFF_SW:_OFF_BW]
            bw_t = cst[:, _OFF_BW:_OFF_FC1B]
            fc1b_t = cst[:, _OFF_FC1B:CONST_F]
            fc1T = zcst[:, _ZOFF_FC1:_ZOFF_S].rearrange("p (k o) -> p k o", k=KC)
            z2s_t = zcst[:, _ZOFF_S:_ZOFF_S + 1]
            z2b_t = zcst[:, _ZOFF_B:_ZOFF_B + 1]

            # pre-warm the sigmoid table set off the critical path
            warm = consts.tile([P, 1], F32)
            nc.scalar.activation(warm[:], cst[:, 0:1], AF.Sigmoid)

            def dma_in(b):
                x_sb = xpool.tile([P, KC, H, W], BF16, tag="x", name=f"x{b}")
                xs = x_d.ap()[b].rearrange("(k p) h w -> p k h w", p=P)
                for kc in range(KC):
                    nc.sync.dma_start(x_sb[:, kc, :, :], xs[:, kc, :, :])
                return x_sb

            def folds(x_sb, y_pre, chunked=False):
                # w-sums -> y_pre[:, :, 0:H]; all bf16 TT adds run 2x_1p
                sw = swpool.tile([P, KC, H, 32], BF16, tag="sw")
                if chunked:
                    # per-chunk L1 so the first folds start as soon as
                    # each input chunk lands (iteration-0 fill)
                    for kc in range(KC):
                        nc.vector.tensor_add(
                            sw[:, kc], x_sb[:, kc, :, 0:32],
                            x_sb[:, kc, :, 32:64])
                else:
                    nc.vector.tensor_add(sw[:], x_sb[:, :, :, 0:32],
                                         x_sb[:, :, :, 32:64])
                nc.vector.tensor_add(sw[:, :, :, 0:16], sw[:, :, :, 0:16],
                                     sw[:, :, :, 16:32])
                nc.vector.tensor_add(sw[:, :, :, 0:8], sw[:, :, :, 0:8],
                                     sw[:, :, :, 8:16])
                nc.vector.tensor_add(sw[:, :, :, 0:4], sw[:, :, :, 0:4],
                                     sw[:, :, :, 4:8])
                nc.vector.tensor_add(sw[:, :, :, 0:2], sw[:, :, :, 0:2],
                                     sw[:, :, :, 2:4])
                nc.vector.tensor_add(y_pre[:, :, 0:H], sw[:, :, :, 0],
                                     sw[:, :, :, 1])
                # h-sums -> y_pre[:, :, H:]
                sh = shpool.tile([P, KC, 32, W], BF16, tag="sh")
                if chunked:
                    for kc in range(KC):
                        nc.vector.tensor_add(
                            sh[:, kc], x_sb[:, kc, 0:32, :],
                            x_sb[:, kc, 32:64, :])
                else:
                    nc.vector.tensor_add(sh[:], x_sb[:, :, 0:32, :],
                                         x_sb[:, :, 32:64, :])
                nc.vector.tensor_add(sh[:, :, 0:16, :], sh[:, :, 0:16, :],
                                     sh[:, :, 16:32, :])
                nc.vector.tensor_add(sh[:, :, 0:8, :], sh[:, :, 0:8, :],
                                     sh[:, :, 8:16, :])
                nc.vector.tensor_add(sh[:, :, 0:4, :], sh[:, :, 0:4, :],
                                     sh[:, :, 4:8, :])
                nc.vector.tensor_add(sh[:, :, 0:2, :], sh[:, :, 0:2, :],
                                     sh[:, :, 2:4, :])
                nc.vector.tensor_add(y_pre[:, :, H:H + W], sh[:, :, 0, :],
                                     sh[:, :, 1, :])

            def chain(y_pre):
                # y = relu((Wy/64 @ y_pre)*sy + by), zraw = row-sums of y
                psum_y = psy.tile([P, KC, H + W], F32, tag="py")
                for oc in range(KC):
                    for kc in range(KC):
                        nc.tensor.matmul(
                            psum_y[:, oc, :],
                            wyT[:, kc, oc * P:(oc + 1) * P],
                            y_pre[:, kc, :],
                            start=(kc == 0), stop=(kc == KC - 1))
                y_sb = ysb_pool.tile([P, KC, H + W], BF16, tag="y")
                zraw = zpool.tile([P, KC, 1], F32, tag="zraw")
                for oc in range(KC):
                    nc.scalar.activation(
                        y_sb[:, oc, :], psum_y[:, oc, :], AF.Relu,
                        bias=by_t[:, oc:oc + 1], scale=sy_t[:, oc:oc + 1],
                        accum_out=zraw[:, oc, :])

                # z chain (tiny, fp32)
                psum_z = psz.tile([R, 1], F32, tag="pz")
                for kc in range(KC):
                    nc.tensor.matmul(
                        psum_z[:], fc0T[:, kc, :], zraw[:, kc, :],
                        start=(kc == 0), stop=(kc == KC - 1))
                z2 = zpool.tile([R, 1], F32, tag="z2")
                nc.scalar.activation(z2[:], psum_z[:], AF.Relu,
                                     bias=z2b_t[:], scale=z2s_t[:])
                # z3 stays in PSUM: the combine tensor_scalar adds
                # psum_z3 and fc1b as its two scalar operands, keeping
                # the ACT queue clear ahead of the replicas
                psum_z3 = psz.tile([P, KC], F32, tag="pz3")
                for oc in range(KC):
                    nc.tensor.matmul(
                        psum_z3[:, oc:oc + 1], fc1T[:, oc, :], z2[:],
                        start=True, stop=True)
                z3 = psum_z3

                # xh / xw
                psum_hw = pshw.tile([P, KC, H + W], F32, tag="phw")
                for oc in range(KC):
                    for kc in range(KC):
                        nc.tensor.matmul(
                            psum_hw[:, oc, 0:H],
                            whT[:, kc, oc * P:(oc + 1) * P],
                            y_sb[:, kc, 0:H],
                            start=(kc == 0), stop=(kc == KC - 1))
                    for kc in range(KC):
                        nc.tensor.matmul(
                            psum_hw[:, oc, H:H + W],
                            wwT[:, kc, oc * P:(oc + 1) * P],
                            y_sb[:, kc, H:H + W],
                            start=(kc == 0), stop=(kc == KC - 1))
                xh = hw_pool.tile([P, KC, H], BF16, tag="xh")
                xw = hw_pool.tile([P, KC, W], BF16, tag="xw")
                reps = {}
                for oc in range(KC):
                    nc.scalar.activation(
                        xh[:, oc, :], psum_hw[:, oc, 0:H], AF.Relu,
                        bias=bh_t[:, oc:oc + 1], scale=sh_t[:, oc:oc + 1])
                    # materialize the xh replica as soon as its epilogue
                    # lands, so DVE's gate muls don't stall on ACT
                    rep = reppool.tile([P, H, W], BF16, tag=f"rep{oc}")
                    nc.scalar.copy(
                        rep[:],
                        xh[:, oc, :].unsqueeze(2).broadcast_to([P, H, W]))
                    reps[oc] = rep
                    nc.scalar.activation(
                        xw[:, oc, :], psum_hw[:, oc, H:H + W], AF.Relu,
                        bias=bw_t[:, oc:oc + 1], scale=sw_t[:, oc:oc + 1])
                return y_sb, zraw, z2, z3, xh, xw, reps

            def emit_gate(st):
                # The xh replica (built by ScalarE inside chain()) makes
                # the outer-product multiply a fully packed bf16 TT that
                # runs in 2x mode on DVE. GpSimd stays idle on purpose:
                # any GpSimd activity degrades DVE 2-port instructions
                # via the shared SBUF port.
                x_sb, xh, xw, z3, reps, b = st
                ls = {}
                for oc in range(KC):
                    l_t = ltpool.tile([P, H, W], BF16, tag="l",
                                      name=f"l{b}_{oc}")
                    for hh in range(NH):
                        h0 = hh * HH
                        nc.vector.tensor_mul(
                            l_t[:, h0:h0 + HH, :],
                            reps[oc][:, h0:h0 + HH, :],
                            xw[:, oc, :].unsqueeze(1)
                              .broadcast_to([P, HH, W]))
                    ls[oc] = l_t
                for oc in range(KC):
                    nc.scalar.activation(ls[oc][:], ls[oc][:], AF.Sigmoid)
                return ls

            def emit_combine(st, ls, split_out=False):
                # x = (sigmoid + z3 + fc1b) * x in place, then store
                x_sb, xh, xw, z3, reps, b = st
                for oc in range(KC):
                    l_t = ls[oc]
                    nc.vector.tensor_scalar(
                        l_t[:], l_t[:], z3[:, oc:oc + 1],
                        fc1b_t[:, oc:oc + 1], op0=OP.add, op1=OP.add)
                    nc.vector.tensor_mul(
                        x_sb[:, oc, :, :], l_t[:], x_sb[:, oc, :, :])
                    if split_out:
                        # drain: halve each store and alternate queues so
                        # the final writeback tail shrinks
                        nc.sync.dma_start(
                            out_d.ap()[b, oc * P:(oc + 1) * P][:, 0:HH, :],
                            x_sb[:, oc, 0:HH, :])
                        nc.scalar.dma_start(
                            out_d.ap()[b, oc * P:(oc + 1) * P][:, HH:H, :],
                            x_sb[:, oc, HH:H, :])
                    else:
                        nc.sync.dma_start(
                            out_d.ap()[b, oc * P:(oc + 1) * P],
                            x_sb[:, oc, :, :])

            prev = None
            prev_ls = None
            x_cur = dma_in(0)
            for b in range(BS):
                x_nxt = dma_in(b + 1) if b + 1 < BS else None
                y_pre = ypre_pool.tile([P, KC, H + W], BF16, tag="ypre")
                folds(x_cur, y_pre, chunked=(b == 0))
                if prev is not None:
                    emit_combine(prev, prev_ls)
                _, _, _, z3, xh, xw, reps = chain(y_pre)
                prev = (x_cur, xh, xw, z3, reps, b)
                prev_ls = emit_gate(prev)
                x_cur = x_nxt
            emit_combine(prev, prev_ls, split_out=True)

    nc.compile()
    return nc


def _pack_consts(Wy, gy, by, Wh, gh, bh, Ww, gw, bw,
                 fc0_w, fc0_b, bn1_g, bn1_b, fc1_w, fc1_b):
    inv = 1.0 / np.sqrt(1.0 + EPS)

    def chunked_T(w):
        # [out, in] -> lhsT tile [p, kc, out]
        return np.ascontiguousarray(
            w.T.reshape(KC, P, C).transpose(1, 0, 2))

    def lanes(v):
        # [C] -> [p, kc]
        return np.ascontiguousarray(v.reshape(KC, P).T)

    wbl = np.empty((P, WBLOB_F), np.float32)
    wbl[:, _OFF_WY:_OFF_WH] = chunked_T(Wy / 64.0).reshape(P, KC * C)
    wbl[:, _OFF_WH:_OFF_WW] = chunked_T(Wh).reshape(P, KC * C)
    wbl[:, _OFF_WW:WBLOB_F] = chunked_T(Ww).reshape(P, KC * C)
    cst = np.empty((P, CONST_F), np.float32)
    # wavelet level-i scale per channel chunk, folded into fc0
    wscale = np.repeat(2.0 ** (np.arange(1, KC + 1) / 2.0) / (H + W), P)
    fc0T_s = (fc0_w * wscale[None, :]).T        # [C, R]
    cst[:, _OFF_FC0:_OFF_SY] = fc0T_s.reshape(KC, P, R).transpose(1, 0, 2) \
                                     .reshape(P, KC * R)
    cst[:, _OFF_SY:_OFF_BY] = lanes(gy * inv)
    cst[:, _OFF_BY:_OFF_SH] = lanes(by)
    cst[:, _OFF_SH:_OFF_BH] = lanes(gh * inv)
    cst[:, _OFF_BH:_OFF_SW] = lanes(bh)
    cst[:, _OFF_SW:_OFF_BW] = lanes(gw * inv)
    cst[:, _OFF_BW:_OFF_FC1B] = lanes(bw)
    cst[:, _OFF_FC1B:CONST_F] = lanes(fc1_b)

    zcst = np.empty((R, ZCONST_F), np.float32)
    zcst[:, _ZOFF_FC1:_ZOFF_S] = fc1_w.T.reshape(R, KC * P)
    z2s = bn1_g * inv
    zcst[:, _ZOFF_S] = z2s
    zcst[:, _ZOFF_B] = fc0_b * z2s + bn1_b
    return wbl.astype(ml_dtypes.bfloat16), cst, zcst


def _get_compiled():
    global _compiled
    if _compiled is None:
        _compiled = _build()
    return _compiled


def kernel(x, Wy, gy, by, Wh, gh, bh, Ww, gw, bw,
           fc0_w, fc0_b, bn1_g, bn1_b, fc1_w, fc1_b,
           _trace=False, _trace_kwargs=None):
    nc = _get_compiled()
    wbl, cst, zcst = _pack_consts(
        np.asarray(Wy, np.float32), np.asarray(gy, np.float32),
        np.asarray(by, np.float32), np.asarray(Wh, np.float32),
        np.asarray(gh, np.float32), np.asarray(bh, np.float32),
        np.asarray(Ww, np.float32), np.asarray(gw, np.float32),
        np.asarray(bw, np.float32), np.asarray(fc0_w, np.float32),
        np.asarray(fc0_b, np.float32), np.asarray(bn1_g, np.float32),
        np.asarray(bn1_b, np.float32), np.asarray(fc1_w, np.float32),
        np.asarray(fc1_b, np.float32))
    x = np.asarray(x, np.float32).astype(ml_dtypes.bfloat16)
    x = np.ascontiguousarray(x)
    in_maps = [
        {"x": x[i * BS:(i + 1) * BS], "wbl": wbl, "cst": cst, "zcst": zcst}
        for i in range(N_CORES)
    ]
    res = run_bass_kernel_spmd(
        nc, in_maps, list(range(N_CORES)),
        trace=_trace, **(_trace_kwargs or {}))
    out = np.concatenate(
        [res.results[i]["out"] for i in range(N_CORES)],
        axis=0).astype(np.float32)
    if _trace:
        return out, res
    return out


# revision 30
# speedup vs baseline: 1.0220x; 1.0084x over previous
"""Trainium2 Bass kernel for the CoordPooling+SFP gate module.

Computation (per batch b):
  y_pre = [sum_w x | sum_h x]                 [C, H+W]   (C=384, H=W=64)
  y  = relu((Wy/64 @ y_pre) * sy + by)        [C, 128]
  xh = relu((Wh @ y[:, :64]) * sh + bh)       [C, 64]
  xw = relu((Ww @ y[:, 64:]) * sw + bw)       [C, 64]
  z_raw[c] = sum_L y[c, :]
  z  = fc1(relu(bn1(fc0(z_raw * wscale))))    [C]
  out = x * sigmoid(xh outer xw) + x * z

Sharding: data-parallel over batch, 4 batches per core on 8 cores.

v2 design (rel-err budget 2e-2, bf16 is ~4e-3):
 - x streams in/out as bf16: halves HBM traffic (the f32 version was
   within 25% of the DMA roofline once compute was fixed).
 - reductions: TENSOR_REDUCE has no fast DVE modes (1 elem/cycle), so
   both pooling sums are binary fold trees of bf16 tensor_tensor adds,
   which hit the packed 2x_1p mode (2 elem/cycle).
 - outer-product muls go to GpSimd (on DVE the stride-0 broadcast
   operand blocks the packed mode, so DVE gains nothing from bf16
   there; GpSimd is otherwise idle).
 - combine is tensor_scalar add (4x mode) + packed TT multiply (2x)
   instead of scalar_tensor_tensor (which has no fast modes at all).
 - ScalarE keeps sigmoid + all CBR epilogues; TensorE does the (tiny)
   matmuls in bf16.
"""

import sys
import numpy as np

for _p in ("/opt/trn_rl_repo", "/root/.axon_site/_ro/trn_rl_repo"):
    if _p not in sys.path:
        sys.path.append(_p)

import ml_dtypes
import concourse.bass as bass
import concourse.tile as tile
from concourse import bacc, mybir
from concourse.bass_utils import run_bass_kernel_spmd

F32 = mybir.dt.float32
BF16 = mybir.dt.bfloat16
AF = mybir.ActivationFunctionType
OP = mybir.AluOpType

N_CORES = 8
B, C, H, W = 32, 384, 64, 64
BS = B // N_CORES          # batches per core
P = 128                    # partitions
KC = C // P                # channel chunks (3)
R = 24                     # gate bottleneck
EPS = 1e-5
HH = 32                    # h-half height for phase-2 units
NH = H // HH               # 2

# const blob layout (free-dim offsets)
_OFF_WY = 0
_OFF_WH = _OFF_WY + KC * C      # 1152
_OFF_WW = _OFF_WH + KC * C
WBLOB_F = _OFF_WW + KC * C      # 3456 (bf16 blob)
_OFF_FC0 = 0
_OFF_SY = _OFF_FC0 + KC * R     # 72
_OFF_BY = _OFF_SY + KC
_OFF_SH = _OFF_BY + KC
_OFF_BH = _OFF_SH + KC
_OFF_SW = _OFF_BH + KC
_OFF_BW = _OFF_SW + KC
_OFF_FC1B = _OFF_BW + KC
CONST_F = _OFF_FC1B + KC        # 93
_ZOFF_FC1 = 0
_ZOFF_S = KC * P                # 384
_ZOFF_B = _ZOFF_S + 1
ZCONST_F = _ZOFF_B + 1          # 386

_compiled = None


def _build():
    nc = bacc.Bacc("TRN2", target_bir_lowering=False, debug=False,
                   num_devices=N_CORES)
    x_d = nc.dram_tensor("x", [BS, C, H, W], BF16, kind="ExternalInput")
    wbl_d = nc.dram_tensor("wbl", [P, WBLOB_F], BF16, kind="ExternalInput")
    cst_d = nc.dram_tensor("cst", [P, CONST_F], F32, kind="ExternalInput")
    zcst_d = nc.dram_tensor("zcst", [R, ZCONST_F], F32, kind="ExternalInput")
    out_d = nc.dram_tensor("out", [BS, C, H, W], BF16, kind="ExternalOutput")

    with tile.TileContext(nc) as tc:
        with (
            tc.tile_pool(name="consts", bufs=1) as consts,
            tc.tile_pool(name="xp", bufs=3) as xpool,
            tc.tile_pool(name="swp", bufs=1) as swpool,
            tc.tile_pool(name="shp", bufs=1) as shpool,
            tc.tile_pool(name="ypre", bufs=2) as ypre_pool,
            tc.tile_pool(name="ysb", bufs=2) as ysb_pool,
            tc.tile_pool(name="hwp", bufs=2) as hw_pool,
            tc.tile_pool(name="zp", bufs=2) as zpool,
            tc.tile_pool(name="rpp", bufs=1) as reppool,
            tc.tile_pool(name="ltp", bufs=6) as ltpool,
            tc.tile_pool(name="psy", bufs=2, space=bass.MemorySpace.PSUM) as psy,
            tc.tile_pool(name="pshw", bufs=2, space=bass.MemorySpace.PSUM) as pshw,
            tc.tile_pool(name="psz", bufs=2, space=bass.MemorySpace.PSUM) as psz,
        ):
            wbl = consts.tile([P, WBLOB_F], BF16)
            nc.scalar.dma_start(wbl[:], wbl_d.ap())
            cst = consts.tile([P, CONST_F], F32)
            nc.scalar.dma_start(cst[:], cst_d.ap())
            zcst = consts.tile([R, ZCONST_F], F32)
            nc.scalar.dma_start(zcst[:], zcst_d.ap())

            wyT = wbl[:, _OFF_WY:_OFF_WH].rearrange("p (k o) -> p k o", k=KC)
            whT = wbl[:, _OFF_WH:_OFF_WW].rearrange("p (k o) -> p k o", k=KC)
            wwT = wbl[:, _OFF_WW:WBLOB_F].rearrange("p (k o) -> p k o", k=KC)
            fc0T = cst[:, _OFF_FC0:_OFF_SY].rearrange("p (k r) -> p k r", k=KC)
            sy_t = cst[:, _OFF_SY:_OFF_BY]
            by_t = cst[:, _OFF_BY:_OFF_SH]
            sh_t = cst[:, _OFF_SH:_OFF_BH]
            bh_t = cst[:, _OFF_BH:_OFF_SW]
            sw_t = cst[:, _OFF_SW:_OFF_BW]
            bw_t = cst[:, _OFF_BW:_OFF_FC1B]
            fc1b_t = cst[:, _OFF_FC1B:CONST_F]
            fc1T = zcst[:, _ZOFF_FC1:_ZOFF_S].rearrange("p (k o) -> p k o", k=KC)
            z2s_t = zcst[:, _ZOFF_S:_ZOFF_S + 1]
            z2b_t = zcst[:, _ZOFF_B:_ZOFF_B + 1]

            # pre-warm the sigmoid table set off the critical path
            warm = consts.tile([P, 1], F32)
            nc.scalar.activation(warm[:], cst[:, 0:1], AF.Sigmoid)

            def dma_in(b, halved=False):
                x_sb = xpool.tile([P, KC, H, W], BF16, tag="x", name=f"x{b}")
                xs = x_d.ap()[b].rearrange("(k p) h w -> p k h w", p=P)
                for kc in range(KC):
                    if halved:
                        # h-half transfers so the first w-folds can
                        # start ~2us sooner on the cold pipeline
                        nc.sync.dma_start(x_sb[:, kc, 0:HH, :],
                                          xs[:, kc, 0:HH, :])
                        nc.sync.dma_start(x_sb[:, kc, HH:H, :],
                                          xs[:, kc, HH:H, :])
                    else:
                        nc.sync.dma_start(x_sb[:, kc, :, :], xs[:, kc, :, :])
                return x_sb

            def folds(x_sb, y_pre, chunked=False):
                # w-sums -> y_pre[:, :, 0:H]; all bf16 TT adds run 2x_1p
                sw = swpool.tile([P, KC, H, 32], BF16, tag="sw")
                if chunked:
                    # per-chunk, per-h-half L1 so the first folds start
                    # as soon as each half-chunk lands (iteration-0 fill)
                    for kc in range(KC):
                        for hh in range(NH):
                            h0 = hh * HH
                            nc.vector.tensor_add(
                                sw[:, kc, h0:h0 + HH, :],
                                x_sb[:, kc, h0:h0 + HH, 0:32],
                                x_sb[:, kc, h0:h0 + HH, 32:64])
                else:
                    nc.vector.tensor_add(sw[:], x_sb[:, :, :, 0:32],
                                         x_sb[:, :, :, 32:64])
                nc.vector.tensor_add(sw[:, :, :, 0:16], sw[:, :, :, 0:16],
                                     sw[:, :, :, 16:32])
                nc.vector.tensor_add(sw[:, :, :, 0:8], sw[:, :, :, 0:8],
                                     sw[:, :, :, 8:16])
                nc.vector.tensor_add(sw[:, :, :, 0:4], sw[:, :, :, 0:4],
                                     sw[:, :, :, 4:8])
                nc.vector.tensor_add(sw[:, :, :, 0:2], sw[:, :, :, 0:2],
                                     sw[:, :, :, 2:4])
                nc.vector.tensor_add(y_pre[:, :, 0:H], sw[:, :, :, 0],
                                     sw[:, :, :, 1])
                # h-sums -> y_pre[:, :, H:]
                sh = shpool.tile([P, KC, 32, W], BF16, tag="sh")
                if chunked:
                    for kc in range(KC):
                        nc.vector.tensor_add(
                            sh[:, kc], x_sb[:, kc, 0:32, :],
                            x_sb[:, kc, 32:64, :])
                else:
                    nc.vector.tensor_add(sh[:], x_sb[:, :, 0:32, :],
                                         x_sb[:, :, 32:64, :])
                nc.vector.tensor_add(sh[:, :, 0:16, :], sh[:, :, 0:16, :],
                                     sh[:, :, 16:32, :])
                nc.vector.tensor_add(sh[:, :, 0:8, :], sh[:, :, 0:8, :],
                                     sh[:, :, 8:16, :])
                nc.vector.tensor_add(sh[:, :, 0:4, :], sh[:, :, 0:4, :],
                                     sh[:, :, 4:8, :])
                nc.vector.tensor_add(sh[:, :, 0:2, :], sh[:, :, 0:2, :],
                                     sh[:, :, 2:4, :])
                nc.vector.tensor_add(y_pre[:, :, H:H + W], sh[:, :, 0, :],
                                     sh[:, :, 1, :])

            def chain(y_pre):
                # y = relu((Wy/64 @ y_pre)*sy + by), zraw = row-sums of y
                psum_y = psy.tile([P, KC, H + W], F32, tag="py")
                for oc in range(KC):
                    for kc in range(KC):
                        nc.tensor.matmul(
                            psum_y[:, oc, :],
                            wyT[:, kc, oc * P:(oc + 1) * P],
                            y_pre[:, kc, :],
                            start=(kc == 0), stop=(kc == KC - 1))
                y_sb = ysb_pool.tile([P, KC, H + W], BF16, tag="y")
                zraw = zpool.tile([P, KC, 1], F32, tag="zraw")
                for oc in range(KC):
                    nc.scalar.activation(
                        y_sb[:, oc, :], psum_y[:, oc, :], AF.Relu,
                        bias=by_t[:, oc:oc + 1], scale=sy_t[:, oc:oc + 1],
                        accum_out=zraw[:, oc, :])

                # z chain (tiny, fp32)
                psum_z = psz.tile([R, 1], F32, tag="pz")
                for kc in range(KC):
                    nc.tensor.matmul(
                        psum_z[:], fc0T[:, kc, :], zraw[:, kc, :],
                        start=(kc == 0), stop=(kc == KC - 1))
                z2 = zpool.tile([R, 1], F32, tag="z2")
                nc.scalar.activation(z2[:], psum_z[:], AF.Relu,
                                     bias=z2b_t[:], scale=z2s_t[:])
                # z3 stays in PSUM: the combine tensor_scalar adds
                # psum_z3 and fc1b as its two scalar operands, keeping
                # the ACT queue clear ahead of the replicas
                psum_z3 = psz.tile([P, KC], F32, tag="pz3")
                for oc in range(KC):
                    nc.tensor.matmul(
                        psum_z3[:, oc:oc + 1], fc1T[:, oc, :], z2[:],
                        start=True, stop=True)
                z3 = psum_z3

                # xh / xw
                psum_hw = pshw.tile([P, KC, H + W], F32, tag="phw")
                for oc in range(KC):
                    for kc in range(KC):
                        nc.tensor.matmul(
                            psum_hw[:, oc, 0:H],
                            whT[:, kc, oc * P:(oc + 1) * P],
                            y_sb[:, kc, 0:H],
                            start=(kc == 0), stop=(kc == KC - 1))
                    for kc in range(KC):
                        nc.tensor.matmul(
                            psum_hw[:, oc, H:H + W],
                            wwT[:, kc, oc * P:(oc + 1) * P],
                            y_sb[:, kc, H:H + W],
                            start=(kc == 0), stop=(kc == KC - 1))
                xh = hw_pool.tile([P, KC, H], BF16, tag="xh")
                xw = hw_pool.tile([P, KC, W], BF16, tag="xw")
                reps = {}
                for oc in range(KC):
                    nc.scalar.activation(
                        xh[:, oc, :], psum_hw[:, oc, 0:H], AF.Relu,
                        bias=bh_t[:, oc:oc + 1], scale=sh_t[:, oc:oc + 1])
                    nc.scalar.activation(
                        xw[:, oc, :], psum_hw[:, oc, H:H + W], AF.Relu,
                        bias=bw_t[:, oc:oc + 1], scale=sw_t[:, oc:oc + 1])
                    # materialize the xh replica in h-halves as soon as
                    # the epilogue lands, so DVE's gate muls stall less
                    rep = reppool.tile([P, H, W], BF16, tag=f"rep{oc}")
                    for hh in range(NH):
                        h0 = hh * HH
                        nc.scalar.copy(
                            rep[:, h0:h0 + HH, :],
                            xh[:, oc, h0:h0 + HH].unsqueeze(2)
                              .broadcast_to([P, HH, W]))
                    reps[oc] = rep
                return y_sb, zraw, z2, z3, xh, xw, reps

            def emit_gate(st):
                # The xh replica (built by ScalarE inside chain()) makes
                # the outer-product multiply a fully packed bf16 TT that
                # runs in 2x mode on DVE. GpSimd stays idle on purpose:
                # any GpSimd activity degrades DVE 2-port instructions
                # via the shared SBUF port.
                x_sb, xh, xw, z3, reps, b = st
                ls = {}
                for oc in range(KC):
                    l_t = ltpool.tile([P, H, W], BF16, tag="l",
                                      name=f"l{b}_{oc}")
                    for hh in range(NH):
                        h0 = hh * HH
                        nc.vector.tensor_mul(
                            l_t[:, h0:h0 + HH, :],
                            reps[oc][:, h0:h0 + HH, :],
                            xw[:, oc, :].unsqueeze(1)
                              .broadcast_to([P, HH, W]))
                    ls[oc] = l_t
                for oc in range(KC):
                    for hh in range(NH):
                        h0 = hh * HH
                        nc.scalar.activation(
                            ls[oc][:, h0:h0 + HH, :],
                            ls[oc][:, h0:h0 + HH, :], AF.Sigmoid)
                return ls

            def emit_combine(st, ls, split_out=False):
                # x = (sigmoid + z3 + fc1b) * x in place, then store
                x_sb, xh, xw, z3, reps, b = st
                for oc in range(KC):
                    l_t = ls[oc]
                    for hh in range(NH):
                        h0 = hh * HH
                        nc.vector.tensor_scalar(
                            l_t[:, h0:h0 + HH, :], l_t[:, h0:h0 + HH, :],
                            z3[:, oc:oc + 1], fc1b_t[:, oc:oc + 1],
                            op0=OP.add, op1=OP.add)
                        nc.vector.tensor_mul(
                            x_sb[:, oc, h0:h0 + HH, :],
                            l_t[:, h0:h0 + HH, :],
                            x_sb[:, oc, h0:h0 + HH, :])
                    if split_out:
                        # drain: halve each store and alternate queues so
                        # the final writeback tail shrinks
                        nc.sync.dma_start(
                            out_d.ap()[b, oc * P:(oc + 1) * P][:, 0:HH, :],
                            x_sb[:, oc, 0:HH, :])
                        nc.scalar.dma_start(
                            out_d.ap()[b, oc * P:(oc + 1) * P][:, HH:H, :],
                            x_sb[:, oc, HH:H, :])
                    else:
                        nc.sync.dma_start(
                            out_d.ap()[b, oc * P:(oc + 1) * P],
                            x_sb[:, oc, :, :])

            prev = None
            prev_ls = None
            x_cur = dma_in(0, halved=True)
            for b in range(BS):
                x_nxt = dma_in(b + 1) if b + 1 < BS else None
                y_pre = ypre_pool.tile([P, KC, H + W], BF16, tag="ypre")
                folds(x_cur, y_pre, chunked=(b == 0))
                if prev is not None:
                    emit_combine(prev, prev_ls)
                _, _, _, z3, xh, xw, reps = chain(y_pre)
                prev = (x_cur, xh, xw, z3, reps, b)
                prev_ls = emit_gate(prev)
                x_cur = x_nxt
            emit_combine(prev, prev_ls, split_out=True)

    nc.compile()
    return nc


def _pack_consts(Wy, gy, by, Wh, gh, bh, Ww, gw, bw,
                 fc0_w, fc0_b, bn1_g, bn1_b, fc1_w, fc1_b):
    inv = 1.0 / np.sqrt(1.0 + EPS)

    def chunked_T(w):
        # [out, in] -> lhsT tile [p, kc, out]
        return np.ascontiguousarray(
            w.T.reshape(KC, P, C).transpose(1, 0, 2))

    def lanes(v):
        # [C] -> [p, kc]
        return np.ascontiguousarray(v.reshape(KC, P).T)

    wbl = np.empty((P, WBLOB_F), np.float32)
    wbl[:, _OFF_WY:_OFF_WH] = chunked_T(Wy / 64.0).reshape(P, KC * C)
    wbl[:, _OFF_WH:_OFF_WW] = chunked_T(Wh).reshape(P, KC * C)
    wbl[:, _OFF_WW:WBLOB_F] = chunked_T(Ww).reshape(P, KC * C)
    cst = np.empty((P, CONST_F), np.float32)
    # wavelet level-i scale per channel chunk, folded into fc0
    wscale = np.repeat(2.0 ** (np.arange(1, KC + 1) / 2.0) / (H + W), P)
    fc0T_s = (fc0_w * wscale[None, :]).T        # [C, R]
    cst[:, _OFF_FC0:_OFF_SY] = fc0T_s.reshape(KC, P, R).transpose(1, 0, 2) \
                                     .reshape(P, KC * R)
    cst[:, _OFF_SY:_OFF_BY] = lanes(gy * inv)
    cst[:, _OFF_BY:_OFF_SH] = lanes(by)
    cst[:, _OFF_SH:_OFF_BH] = lanes(gh * inv)
    cst[:, _OFF_BH:_OFF_SW] = lanes(bh)
    cst[:, _OFF_SW:_OFF_BW] = lanes(gw * inv)
    cst[:, _OFF_BW:_OFF_FC1B] = lanes(bw)
    cst[:, _OFF_FC1B:CONST_F] = lanes(fc1_b)

    zcst = np.empty((R, ZCONST_F), np.float32)
    zcst[:, _ZOFF_FC1:_ZOFF_S] = fc1_w.T.reshape(R, KC * P)
    z2s = bn1_g * inv
    zcst[:, _ZOFF_S] = z2s
    zcst[:, _ZOFF_B] = fc0_b * z2s + bn1_b
    return wbl.astype(ml_dtypes.bfloat16), cst, zcst


def _get_compiled():
    global _compiled
    if _compiled is None:
        _compiled = _build()
    return _compiled


def kernel(x, Wy, gy, by, Wh, gh, bh, Ww, gw, bw,
           fc0_w, fc0_b, bn1_g, bn1_b, fc1_w, fc1_b,
           _trace=False, _trace_kwargs=None):
    nc = _get_compiled()
    wbl, cst, zcst = _pack_consts(
        np.asarray(Wy, np.float32), np.asarray(gy, np.float32),
        np.asarray(by, np.float32), np.asarray(Wh, np.float32),
        np.asarray(gh, np.float32), np.asarray(bh, np.float32),
        np.asarray(Ww, np.float32), np.asarray(gw, np.float32),
        np.asarray(bw, np.float32), np.asarray(fc0_w, np.float32),
        np.asarray(fc0_b, np.float32), np.asarray(bn1_g, np.float32),
        np.asarray(bn1_b, np.float32), np.asarray(fc1_w, np.float32),
        np.asarray(fc1_b, np.float32))
    x = np.asarray(x, np.float32).astype(ml_dtypes.bfloat16)
    x = np.ascontiguousarray(x)
    in_maps = [
        {"x": x[i * BS:(i + 1) * BS], "wbl": wbl, "cst": cst, "zcst": zcst}
        for i in range(N_CORES)
    ]
    res = run_bass_kernel_spmd(
        nc, in_maps, list(range(N_CORES)),
        trace=_trace, **(_trace_kwargs or {}))
    out = np.concatenate(
        [res.results[i]["out"] for i in range(N_CORES)],
        axis=0).astype(np.float32)
    if _trace:
        return out, res
    return out
